# revision 5
# baseline (speedup 1.0000x reference)
"""Trainium2 Bass kernel for CrossModalFusion (B=4, C=64, H=W=64, N=4096).

Reference computation (per sample b, with x reshaped to [C, N]):
    q = wq @ xo + bq          [8, N]
    k = wk @ xs + bk          [8, N]
    v = wv @ xs + bv          [64, N]
    S[n, m]  = q[:, n] . k[:, m]
    attn     = softmax_m(S)
    out      = gamma * (v @ attn^T) + x_opt

Sharding: 8 cores = 4 batch samples x 2 halves of the query (n) axis.
Each core computes output rows [64, 2048] for its (sample, n-half); no
cross-core communication is needed.

Per-core dataflow. The steady-state bottleneck is the ACT (scalar) engine
exp stream: 8.4M score elements -> 65536 lane-cycles = 54.6us payload at
1.2 GHz, plus ~293ns fixed overhead per ACTIVATE.  Everything else is
organized to keep ACT 100% fed and everything off its critical path:
  - biases folded into augmented weights on the host (ones-row trick);
    gamma folded into wv/bv, so the attention output comes out pre-scaled
    and the softmax denominator column stays unscaled.
  - scores computed TRANSPOSED (S^T[m, n]) in quads: 4 concurrent rank-8
    matmuls in the four 32-row PE groups (k/q replicated at partition
    offsets 0/32/64/96 via widened weight matmuls).
  - PSUM layout: score ring 2 x [128, 1536] (3 banks each) + av
    accumulator 2 x [CA, 512] (1 bank each) = 8 banks.  exp batches are
    [128, 1536] (11 ACTIVATEs per n-tile instead of 16 x 1024).
  - AV matmuls are single 128-deep accumulating matmuls (vT block
    stationary, exp'd scores moving) into one av tile; vT carries a ones
    column so row C accumulates the softmax denominator for free.
  - normalize: reciprocal_approx_fast (custom DVE op, ~5x faster than the
    iterative reciprocal) on the denominator row straight out of PSUM,
    broadcast via a rank-1 PE matmul, multiply+residual-add on DVE.  The
    av PSUM tile is read directly (no drain copy).
  - PE warm-up: a short burst of zero matmuls at kernel start keeps the
    PE busy through the DMA fill so the HAM clock gate reaches 2.4 GHz
    before the steady state (cold PE at 1.2 GHz would out-bottleneck ACT).
  - DMA triggers split across the Sync and GpSimd queues (a DMA_DIRECT2D
    costs ~0.5-1us of queue issue time); lead chunks are 512 cols so the
    first score matmul fires early.  vT casts run on GpSimd, k/q casts on
    DVE, so neither blocks the other in n-tile 0.
"""

import os
import sys

import numpy as np

for _p in ("/opt/trn_rl_repo", "/root/.axon_site/_ro/trn_rl_repo"):
    if os.path.isdir(_p) and _p not in sys.path:
        sys.path.insert(0, _p)

import concourse.bass as bass
import concourse.mybir as mybir
import concourse.tile as tile
from concourse import bacc
from concourse.bass_utils import run_bass_kernel_spmd

F32 = mybir.dt.float32
F32R = mybir.dt.float32r
BF16 = mybir.dt.bfloat16
AF = mybir.ActivationFunctionType

B, C, HH, WW = 4, 64, 64, 64
N = HH * WW            # 4096 key/query positions
D = 8                  # q/k channel count
CA = C + 1             # augmented channel dim (ones row / denominator row)
NCORES = 8
NL = N // 2            # query rows per core
NT = 512               # n-tile (PSUM bank width in fp32)
MB = 128               # m-block (PE partition width)
N_NT = NL // NT        # 4 n-tiles per core
N_MB = N // MB         # 32 m-blocks
EB = 3                 # m-blocks per exp batch (st ring slot = 3 banks)
N_ET = (N_MB + EB - 1) // EB   # 11 e-tiles per n-tile (10x3 + 1x2)
WREP = 104             # k/q replication width (4 copies at offsets 0/32/64/96)
WCOLS = 2 * WREP + CA  # combined weight tensor columns (wk4 | wq4 | wv)
E_DTYPE = F32R         # exp output / AV moving operand dtype
N_DUMMY = 5            # PE warm-up matmuls at kernel start


def build_program(repeat: int = 1) -> bass.Bass:
    nc = bacc.Bacc("TRN2", target_bir_lowering=False, num_devices=NCORES)
    xs_d = nc.declare_dram_parameter("xs_bf", [CA, N], BF16, isOutput=False)
    xo_d = nc.declare_dram_parameter("xo_bf", [CA, NL], BF16, isOutput=False)
    xr_d = nc.declare_dram_parameter("xores", [C, NL], F32, isOutput=False)
    w_d = nc.declare_dram_parameter("w_bf", [CA, WCOLS], BF16, isOutput=False)
    out_d = nc.declare_dram_parameter("out", [C, NL], F32, isOutput=True)

    with tile.TileContext(nc) as tc:
      for _rep in range(repeat):
        with (
            tc.tile_pool(name="const", bufs=1) as cp,
            tc.tile_pool(name="st_ps", bufs=2, space="PSUM") as st_pool,
            tc.tile_pool(name="av_ps", bufs=2, space="PSUM") as av_pool,
            tc.tile_pool(name="e_sb", bufs=3) as e_pool,
            tc.tile_pool(name="o_sb", bufs=2) as o_pool,
            tc.tile_pool(name="sm_sb", bufs=2) as sm_pool,
        ):
            # -- warm-up source (zeros) + PE warm-up burst ---------------
            zsrc = cp.tile([MB, 16 + NT], BF16)
            nc.vector.memset(zsrc[:], 0.0)
            ones_sb = cp.tile([1, C], BF16)
            nc.vector.memset(ones_sb[:], 1.0)
            for dmy in range(N_DUMMY):
                dtile = av_pool.tile([MB, NT], F32, tag="av", name=f"dmy{dmy}")
                nc.tensor.matmul(
                    dtile[0:16, :], zsrc[:, 0:16], zsrc[:, 16 : 16 + NT],
                    start=True, stop=True,
                )

            # -- input DMAs: lead chunks first, split across 2 queues ----
            xs_sb = cp.tile([CA, N], BF16)
            xo_sb = cp.tile([CA, NL], BF16)
            w_sb = cp.tile([CA, WCOLS], BF16)
            xr_sb = cp.tile([C, NL], F32)
            wk_sb = w_sb[:, 0:WREP]
            wq_sb = w_sb[:, WREP : 2 * WREP]
            wv_sb = w_sb[:, 2 * WREP : WCOLS]
            # sync queue: k-path lead (xs chunk 0 + weights), then bulk xs
            nc.sync.dma_start(xs_sb[:, 0:NT], xs_d[:, 0:NT])
            nc.sync.dma_start(w_sb[:], w_d[:])
            nc.sync.dma_start(xs_sb[:, NT : NT + 1536], xs_d[:, NT : NT + 1536])
            nc.sync.dma_start(xs_sb[:, 2048:N], xs_d[:, 2048:N])
            # gpsimd queue: q-path lead (xo chunk 0), residual, bulk xo
            nc.gpsimd.dma_start(xo_sb[:, 0:NT], xo_d[:, 0:NT])
            nc.gpsimd.dma_start(xo_sb[:, NT:NL], xo_d[:, NT:NL])
            nc.gpsimd.dma_start(xr_sb[:], xr_d[:])

            # q/k replicated at partition offsets 0/32/64/96 (score row
            # groups); vT blocks [128, 65] with trailing ones column.
            q_rep = cp.tile([WREP, NL], BF16)
            k_rep = cp.tile([WREP, N], BF16)
            vT = cp.tile([MB, N_MB * CA], E_DTYPE)

            # w_sb holds 4 copies of the weights at col offsets 0/32/64/96,
            # so one matmul lands k/q at all four partition groups and one
            # CAST moves them to SBUF -- no replication DMAs.
            def prep_k_chunk(c):
                kp = st_pool.tile([WREP, NT], F32, tag="st", name=f"kp{c}")
                nc.tensor.matmul(
                    kp[:], wk_sb[:], xs_sb[:, c * NT : (c + 1) * NT],
                    start=True, stop=True,
                )
                nc.vector.tensor_copy(k_rep[:, c * NT : (c + 1) * NT], kp[:])

            def prep_q_chunk(c):
                qp = st_pool.tile([WREP, NT], F32, tag="st", name=f"qp{c}")
                nc.tensor.matmul(
                    qp[:], wq_sb[:], xo_sb[:, c * NT : (c + 1) * NT],
                    start=True, stop=True,
                )
                nc.vector.tensor_copy(q_rep[:, c * NT : (c + 1) * NT], qp[:])

            def prep_vt_quad(p):
                # 4 vT blocks through one PSUM slot, one batched cast
                vp = st_pool.tile([MB, 4 * CA], F32, tag="st", name=f"vp{p}")
                for i in range(4):
                    mb = 4 * p + i
                    nc.tensor.matmul(
                        vp[:, i * CA : (i + 1) * CA],
                        xs_sb[:, mb * MB : (mb + 1) * MB], wv_sb[:],
                        start=True, stop=True,
                    )
                nc.vector.tensor_copy(vT[:, 4 * p * CA : (4 * p + 4) * CA], vp[:])

            prep_k_chunk(0)
            prep_q_chunk(0)

            pending_norm = []
            norm_state = {}

            def norm_a(nt, av):
                # reciprocal of the denominator row, straight out of PSUM
                r = sm_pool.tile([1, NT], F32, tag="r", name=f"r{nt}")
                nc.vector.reciprocal_approx_fast(r[:], av[C:CA, :])
                rb = sm_pool.tile([1, NT], BF16, tag="rb", name=f"rb{nt}")
                nc.vector.tensor_copy(rb[:], r[:])
                avS = o_pool.tile([C, NT], F32, tag="avS", name=f"avS{nt}")
                nc.vector.tensor_copy(avS[:], av[0:C, :])
                norm_state[nt] = (rb, avS)

            def norm_b(nt, av):
                rb, avS = norm_state.pop(nt)
                n0b, n1b = nt * NT, (nt + 1) * NT
                bc = st_pool.tile([C, NT], F32, tag="st", name=f"bc{nt}")
                nc.tensor.matmul(bc[:], ones_sb[:], rb[:], start=True, stop=True)
                om = o_pool.tile([C, NT], F32, tag="om", name=f"om{nt}")
                nc.vector.tensor_mul(om[:], bc[:], avS[:])
                o = o_pool.tile([C, NT], F32, tag="o", name=f"o{nt}")
                nc.vector.tensor_add(o[:], om[:], xr_sb[:, n0b:n1b])
                nc.sync.dma_start(out_d[:, n0b:n1b], o[:])

            for nt in range(N_NT):
                n0, n1 = nt * NT, (nt + 1) * NT
                av = av_pool.tile([CA, NT], F32, tag="av", name=f"av{nt}")

                def emit_av(t, e_t, av=av):
                    mbs = range(EB * t, min(EB * t + EB, N_MB))
                    for i, mb in enumerate(mbs):
                        nc.tensor.matmul(
                            av[:],
                            vT[:, mb * CA : (mb + 1) * CA],
                            e_t[:, i * NT : (i + 1) * NT],
                            start=(mb == 0), stop=(mb == N_MB - 1),
                        )

                # e-tile waves: 3 score matmuls (PE row group = mb%4, so any
                # run of 4 consecutive m-blocks executes concurrently), one
                # [128, width*512] exp, AV of the previous e-tile, prep/norm
                # hooks at tile boundaries only (keeps the st ring ordered).
                pend = []
                for t in range(N_ET):
                    width = min(EB, N_MB - EB * t)
                    stt = st_pool.tile(
                        [MB, width * NT], F32, tag="st", name=f"st{nt}_{t}"
                    )
                    for bi in range(width):
                        mb = EB * t + bi
                        rg = 32 * (mb % 4)
                        nc.tensor.matmul(
                            stt[:, bi * NT : (bi + 1) * NT],
                            k_rep[rg : rg + D, mb * MB : (mb + 1) * MB],
                            q_rep[rg : rg + D, n0:n1],
                            start=True, stop=True,
                            tile_position=(rg, 0),
                        )
                    e_t = e_pool.tile(
                        [MB, width * NT], E_DTYPE, tag="e", name=f"e{nt}_{t}"
                    )
                    nc.scalar.activation(e_t[:], stt[:], AF.Exp)
                    pend.append((t, e_t))
                    if pending_norm and t == 1:
                        norm_a(*pending_norm[0])
                    if pending_norm and t == 4:
                        norm_b(*pending_norm.pop(0))
                    while len(pend) > 1:
                        emit_av(*pend.pop(0))
                    if nt == 0:
                        if t < 7:
                            prep_k_chunk(t + 1)
                        if t in (2, 5, 8) and t // 3 + 1 < N_NT:
                            prep_q_chunk(t // 3 + 1)
                        if t < 8:
                            prep_vt_quad(t)
                while pend:
                    emit_av(*pend.pop(0))

                pending_norm.append((nt, av))
                if nt == N_NT - 1:
                    while pending_norm:
                        norm_a(*pending_norm[0])
                        norm_b(*pending_norm.pop(0))
    nc.compile()
    return nc


_NC = None


def _get_nc() -> bass.Bass:
    global _NC
    if _NC is None:
        _NC = build_program()
    return _NC


def _to_bf16(a: np.ndarray) -> np.ndarray:
    """Round-to-nearest-even fp32 -> bf16 (ml_dtypes view)."""
    import ml_dtypes

    u = np.ascontiguousarray(a, np.float32).view(np.uint32)
    rounded = ((u + 0x7FFF + ((u >> 16) & 1)) >> 16).astype(np.uint16)
    return rounded.view(ml_dtypes.bfloat16)


def make_in_maps(x_opt, x_sar, wq, bq, wk, bk, wv, bv, gamma):
    f = np.float32
    x_opt = np.asarray(x_opt, f).reshape(B, C, N)
    x_sar = np.asarray(x_sar, f).reshape(B, C, N)
    g = float(np.asarray(gamma, f).reshape(()))
    wq_aug = np.concatenate([np.asarray(wq, f).T, np.asarray(bq, f)[None, :]], axis=0)
    wk_aug = np.concatenate([np.asarray(wk, f).T, np.asarray(bk, f)[None, :]], axis=0)
    # gamma folded into v (weights AND bias); denominator column stays 1.
    wv_aug = np.zeros((CA, CA), f)
    wv_aug[:C, :C] = np.asarray(wv, f).T * g
    wv_aug[C, :C] = np.asarray(bv, f) * g
    wv_aug[C, C] = 1.0
    w_all = np.zeros((CA, WCOLS), f)
    for gidx in range(4):
        w_all[:, 32 * gidx : 32 * gidx + D] = wk_aug
        w_all[:, WREP + 32 * gidx : WREP + 32 * gidx + D] = wq_aug
    w_all[:, 2 * WREP : WCOLS] = wv_aug
    w_bf = _to_bf16(w_all)
    ones_n = np.ones((1, N), f)
    maps = []
    for core in range(NCORES):
        b, h = divmod(core, 2)
        xo_aug = np.concatenate(
            [x_opt[b, :, h * NL : (h + 1) * NL], ones_n[:, :NL]], axis=0
        )
        xs_aug = np.concatenate([x_sar[b], ones_n], axis=0)
        maps.append(
            {
                "xo_bf": _to_bf16(xo_aug),
                "xs_bf": _to_bf16(xs_aug),
                "xores": np.ascontiguousarray(x_opt[b, :, h * NL : (h + 1) * NL]),
                "w_bf": w_bf,
            }
        )
    return maps


def assemble_out(results) -> np.ndarray:
    out = np.empty((B, C, N), np.float32)
    for core in range(NCORES):
        b, h = divmod(core, 2)
        out[b, :, h * NL : (h + 1) * NL] = results[core]["out"]
    return out.reshape(B, C, HH, WW)


def kernel(**inputs) -> np.ndarray:
    nc = _get_nc()
    maps = make_in_maps(**inputs)
    res = run_bass_kernel_spmd(nc, maps, list(range(NCORES)))
    return assemble_out(res.results)
